# revision 3
# baseline (speedup 1.0000x reference)
"""MoE FFN (8 experts, top-2) on 8 Trainium2 NeuronCores.

Hidden-dimension sharding for perfect load balance: every core processes ALL
16384 (token, expert) pairs, but only H/8 = 512 of each expert's 4096 hidden
units. Per-core work is exactly 2048 token-equivalents regardless of routing
skew (per-expert counts vary +-130, which under expert parallelism pads every
core to the max count). Each core holds H-slice weights of all 8 experts
(16.8 MB bf16, SBUF-resident) and emits a partial output in bf16; the host
sums the 8 partials and applies the combine weights. b2 is fed as b2/8 so the
partial sum reproduces the bias exactly once.

The token stream is sorted by expert; tile boundaries are specialized to the
routing at build time (compile cache keyed on the per-expert counts), so tiles
never straddle an expert boundary and no capacity padding exists anywhere.

On-device layout: all matmul operands keep the contraction dim on SBUF
partitions. PSUM accumulates in f32; layer-1 bias rides the gelu on ScalarE,
layer-2 bias (pre-divided by 8) is fused into the PSUM eviction on VectorE,
which also downcasts the partial to bf16.
"""

import numpy as np
import ml_dtypes

N_EXPERTS = 8
TOP_K = 2
C = 1024
H = 4096
HS = H // N_EXPERTS      # per-core hidden slice
P = 128
T_TILE = 512
KO1 = C // P             # 8 contraction chunks for layer 1
KO2 = HS // P            # 4 contraction chunks for layer 2
MO1 = HS // P            # 4 output chunks for layer 1
CO2 = C // P             # 8 output chunks for layer 2
TP = 16384               # total (token, expert) pairs: 8192 tokens * top-2

_nc_cache = {}


def _tile_plan(counts):
    """Balanced single-expert tiles over the expert-sorted pair stream."""
    tiles = []
    t0 = 0
    for e, c in enumerate(counts):
        if c == 0:
            continue
        k = -(-c // T_TILE)
        for i in range(k):
            T = c // k + (1 if i < c % k else 0)
            tiles.append((e, t0, T))
            t0 += T
    assert t0 == sum(counts)
    return tiles


def _build_nc(counts):
    import concourse.mybir as mybir
    import concourse.tile as tile
    from concourse import bacc

    bf16 = mybir.dt.bfloat16
    f32 = mybir.dt.float32

    nc = bacc.Bacc()
    xt = nc.dram_tensor("xt", [C, TP], bf16, kind="ExternalInput")
    w1 = [
        nc.dram_tensor(f"w1_{e}", [C, HS], bf16, kind="ExternalInput")
        for e in range(N_EXPERTS)
    ]
    w2 = [
        nc.dram_tensor(f"w2_{e}", [HS, C], bf16, kind="ExternalInput")
        for e in range(N_EXPERTS)
    ]
    # biases host-pre-swizzled: [P, n_chunks], partition-major per chunk
    b1 = [
        nc.dram_tensor(f"b1_{e}", [P, MO1], f32, kind="ExternalInput")
        for e in range(N_EXPERTS)
    ]
    b2 = [
        nc.dram_tensor(f"b2_{e}", [P, CO2], f32, kind="ExternalInput")
        for e in range(N_EXPERTS)
    ]
    yt = nc.dram_tensor("yt", [C, TP], bf16, kind="ExternalOutput")

    xt_r = xt.rearrange("(ko ki) t -> ki ko t", ki=P)
    w1_r = [w.rearrange("(ko ki) h -> ki ko h", ki=P) for w in w1]
    w2_r = [w.rearrange("(ko ki) c -> ki ko c", ki=P) for w in w2]
    yt_r = yt.rearrange("(co p) t -> p co t", p=P)

    tiles = _tile_plan(counts)
    used = sorted({e for e, _, _ in tiles})
    gelu = mybir.ActivationFunctionType.Gelu_apprx_tanh

    with tile.TileContext(nc) as tc:
        with (
            tc.tile_pool(name="const", bufs=1) as const,
            tc.tile_pool(name="xp", bufs=3) as xp,
            tc.tile_pool(name="gp", bufs=2) as gp,
            tc.tile_pool(name="yp", bufs=4) as yp,
            tc.tile_pool(name="psum", bufs=8, space="PSUM") as psum,
        ):
            w1_sb = {
                e: const.tile([P, KO1, HS], bf16, tag=f"w1_{e}", name=f"w1s{e}")
                for e in used
            }
            w2_sb = {
                e: const.tile([P, KO2, C], bf16, tag=f"w2_{e}", name=f"w2s{e}")
                for e in used
            }
            b1_sb = {
                e: const.tile([P, MO1], f32, tag=f"b1_{e}", name=f"b1s{e}")
                for e in used
            }
            b2_sb = {
                e: const.tile([P, CO2], f32, tag=f"b2_{e}", name=f"b2s{e}")
                for e in used
            }

            # The HWDGE stream drains serially in program order: issue loads in
            # exactly first-consumption order. Tile 0 needs x(t0) and the first
            # expert's w1 immediately; its w2 ~7us later; expert i's slices are
            # not needed until ~i*56us in, so a simple sequential stream keeps
            # far ahead of consumption after the first expert.
            e0, t0_, T0 = tiles[0]
            x_tiles = {}
            x_tiles[0] = xp.tile([P, KO1, T_TILE], bf16, tag="x", name="x0")
            for ko in range(KO1):
                nc.sync.dma_start(
                    x_tiles[0][:, ko : ko + 1, :T0], xt_r[:, ko : ko + 1, t0_ : t0_ + T0]
                )
                nc.sync.dma_start(
                    w1_sb[e0][:, ko : ko + 1, 0 : HS // 2],
                    w1_r[e0][:, ko : ko + 1, 0 : HS // 2],
                )
            for ko in range(KO1):
                nc.sync.dma_start(
                    w1_sb[e0][:, ko : ko + 1, HS // 2 :],
                    w1_r[e0][:, ko : ko + 1, HS // 2 :],
                )
            for e in used:
                nc.sync.dma_start(b1_sb[e][:], b1[e][:])
                nc.sync.dma_start(b2_sb[e][:], b2[e][:])
            for ko in range(KO2):
                nc.sync.dma_start(
                    w2_sb[e0][:, ko : ko + 1, :], w2_r[e0][:, ko : ko + 1, :]
                )
            for e in used:
                if e == e0:
                    continue
                for ko in range(KO1):
                    nc.sync.dma_start(
                        w1_sb[e][:, ko : ko + 1, :], w1_r[e][:, ko : ko + 1, :]
                    )
                for ko in range(KO2):
                    nc.sync.dma_start(
                        w2_sb[e][:, ko : ko + 1, :], w2_r[e][:, ko : ko + 1, :]
                    )

            for ti, (e, t0, T) in enumerate(tiles):
                if ti + 1 < len(tiles):
                    ne, nt0, nt = tiles[ti + 1]
                    x_tiles[ti + 1] = xp.tile(
                        [P, KO1, T_TILE], bf16, tag="x", name=f"x{ti + 1}"
                    )
                    nc.sync.dma_start(
                        x_tiles[ti + 1][:, :, :nt], xt_r[:, :, nt0 : nt0 + nt]
                    )
                x_sb = x_tiles.pop(ti)
                g_sb = gp.tile([P, KO2, T_TILE], bf16, tag="g")
                for m in range(MO1):
                    ph = psum.tile([P, T_TILE], mybir.dt.float32, tag="ps")
                    for ko in range(KO1):
                        nc.tensor.matmul(
                            ph[:, :T],
                            w1_sb[e][:, ko, m * P : (m + 1) * P],
                            x_sb[:, ko, :T],
                            start=(ko == 0),
                            stop=(ko == KO1 - 1),
                        )
                    nc.scalar.activation(
                        g_sb[:, m, :T], ph[:, :T], gelu, bias=b1_sb[e][:, m : m + 1]
                    )
                for co in range(CO2):
                    py = psum.tile([P, T_TILE], mybir.dt.float32, tag="ps")
                    for ho in range(KO2):
                        nc.tensor.matmul(
                            py[:, :T],
                            w2_sb[e][:, ho, co * P : (co + 1) * P],
                            g_sb[:, ho, :T],
                            start=(ho == 0),
                            stop=(ho == KO2 - 1),
                        )
                    y_sb = yp.tile([P, T_TILE], bf16, tag="y")
                    nc.vector.tensor_scalar_add(
                        y_sb[:, :T], py[:, :T], b2_sb[e][:, co : co + 1]
                    )
                    nc.sync.dma_start(yt_r[:, co, t0 : t0 + T], y_sb[:, :T])
    nc.finalize()
    return nc


def _route(flat_f32: np.ndarray, gate_w: np.ndarray):
    """Router, bit-matching the reference's jax ops (same env/backend)."""
    import jax
    import jax.numpy as jnp

    logits = jnp.asarray(flat_f32) @ jnp.asarray(gate_w).T
    probs = jax.nn.softmax(logits, axis=-1)
    top_p, top_i = jax.lax.top_k(probs, TOP_K)
    weights = top_p / (jnp.sum(top_p, axis=-1, keepdims=True) + 1e-8)
    return np.asarray(top_i), np.asarray(weights)


# results of the last device run, for test harness introspection
last_result = None


def _ensure_ntff_hook():
    """bass_utils' trace path imports antenv.axon_hooks, which the agent
    image's antenv lacks. Build the hook from trn_agent_boot's ctypes
    shim and inject a stand-in module."""
    import sys
    import types

    if "antenv.axon_hooks" in sys.modules:
        return
    try:
        from trn_agent_boot.trn_boot import _ntff_profile_via_ctypes

        hook = _ntff_profile_via_ctypes("/opt/axon/libaxon_pjrt.so")
    except Exception:
        hook = None
    m = types.ModuleType("antenv.axon_hooks")
    m.get_axon_ntff_profile_hook = lambda: hook
    m.set_axon_ntff_profile_hook = lambda h: None
    sys.modules["antenv.axon_hooks"] = m


def kernel(x, gate_w, w1, b1, w2, b2):
    from concourse.bass_utils import run_bass_kernel_spmd

    x = np.asarray(x)
    B, N, _ = x.shape
    flat = np.ascontiguousarray(x.reshape(-1, C), dtype=np.float32)
    T = flat.shape[0]
    assert T * TOP_K == TP

    top_i, weights = _route(flat, np.asarray(gate_w, dtype=np.float32))

    # expert-sorted pair stream
    tok_e = []
    wgt_e = []
    for e in range(N_EXPERTS):
        rows, cols = np.nonzero(top_i == e)
        tok_e.append(rows.astype(np.int64))
        wgt_e.append(weights[rows, cols].astype(np.float32))
    counts = tuple(len(i) for i in tok_e)
    pair_tok = np.concatenate(tok_e)
    pair_w = np.concatenate(wgt_e)

    nc = _nc_cache.get(counts)
    if nc is None:
        nc = _build_nc(counts)
        _nc_cache[counts] = nc

    bf16 = ml_dtypes.bfloat16
    xs = np.ascontiguousarray(flat[pair_tok].T).astype(bf16)  # [C, TP]
    w1 = np.asarray(w1, dtype=np.float32)
    w2 = np.asarray(w2, dtype=np.float32)
    b1 = np.asarray(b1, dtype=np.float32)
    b2 = np.asarray(b2, dtype=np.float32)

    in_maps = []
    for k in range(N_EXPERTS):
        hs = slice(k * HS, (k + 1) * HS)
        im = {"xt": xs}
        for e in range(N_EXPERTS):
            im[f"w1_{e}"] = np.ascontiguousarray(w1[e, hs, :].T).astype(bf16)
            im[f"w2_{e}"] = np.ascontiguousarray(w2[e, :, hs].T).astype(bf16)
            im[f"b1_{e}"] = np.ascontiguousarray(
                b1[e, hs].reshape(MO1, P).T
            )
            im[f"b2_{e}"] = np.ascontiguousarray(
                (b2[e] / N_EXPERTS).reshape(CO2, P).T
            )
        in_maps.append(im)

    import os

    trace = bool(int(os.environ.get("MOE_TRACE", "0")))
    if trace:
        _ensure_ntff_hook()

    global last_result
    res = run_bass_kernel_spmd(
        nc,
        in_maps,
        core_ids=list(range(N_EXPERTS)),
        trace=trace,
    )
    last_result = res

    ysum = np.zeros((C, TP), dtype=np.float32)
    for k in range(N_EXPERTS):
        ysum += res.results[k]["yt"].astype(np.float32)
    contrib = (ysum * pair_w[None, :]).T  # [TP, C]
    out = np.zeros((T, C), dtype=np.float32)
    np.add.at(out, pair_tok, contrib)
    return out.reshape(B, N, C)
